# revision 37
# baseline (speedup 1.0000x reference)
"""Trainium2 Bass kernel for DiffusionNet3D.

Math (see reference): state (B,48,48,48), T=64 steps of
  state = tanh(sum_{27 taps} shift(state) * per-voxel-filter),
then adaptive-pool the last depth plane into (B,10) logits.

Structure exploited:
  * state_0 is nonzero only in plane d=0 and the readout needs only plane
    d=47 at step T: at step t only planes [t-16, t+1] matter -> an <=18
    plane sliding window (3.6x compute cut vs the full 48-plane volume).
  * Spatially-varying weights make this elementwise work: DVE computes the
    27 shifted products; the TensorEngine accumulates them into PSUM with
    identity matmuls; ScalarE applies tanh.
  * Sharding: pure batch across the 8 cores (B=32 -> 4/core, no inter-core
    comm). On-chip: partitions = 128 (hc,wc) chunks of the 48x48 plane
    (3x6 each); free dim = (d-window, h, w, b) with d shifts as free-dim
    offsets. h/w halo cells are exchanged across partitions each step with
    shift-matrix matmuls on the otherwise idle PE (PSUM -> halo cells via
    ScalarE copies). Everything lives in SBUF; zero DMA inside the loop.

Toolchain constraint (axon/neuronxcc walrus): each instruction may carry at
most ONE semaphore wait. Tile emits one wait per dependency proc, including
redundant same-engine ones. The kernel is therefore structured so that every
instruction depends on exactly one foreign engine:
  * one concatenated input DMA (single DMA-queue semaphore),
  * "marker" matmuls + 1-elem DVE "absorber" reads convert the product-pool
    WAR (DVE write after PE read) into an already-observed PE tick,
  * a 1-elem ACT "cover" read of the last product each step hands the DVE
    clock to ScalarE so tanh's WAR on the previous step's products is free,
  * a post-pass strips the redundant same-engine waits (engines execute
    in order) and spreads the kernel-tail drain's waits across the other
    engines' drains (whose waits are trivial >=0 barrier placeholders).
"""

import numpy as np
from contextlib import ExitStack

import concourse.bass as bass
import concourse.mybir as mybir
import concourse.tile as tile
from concourse.bass import AP
from concourse.bass_utils import run_bass_kernel_spmd
from concourse.tile_rust import add_dep_helper

B, D, H, W = 32, 48, 48, 48
NUM_CLASSES = 10
NCORES = 8
BL = B // NCORES            # 4 batch per core
NHC, NWC = 16, 8            # h,w chunk grid -> 128 partitions
HI, WI = H // NHC, W // NWC  # 3, 6 chunk shape
P = NHC * NWC               # 128
DP, HPAD, WPAD = D + 2, HI + 2, WI + 2   # 50, 5, 8
NSTATE = DP * HPAD * WPAD * BL           # 8000 elems/partition
NPLANE = HI * WI * BL                    # 72 interior elems per plane
NFILT = D * HI * WI * 27                 # 23328 filter elems/partition
F32 = mybir.dt.float32

WMAX = 18                   # max active-window planes
GRP = 2                     # taps per product-pool slot
PGN = GRP * WMAX * NPLANE   # product slot payload elems (2592)

# cbuf layout: [filters | mat_id | mat_n | mat_s | mat_w | mat_e | x]
MAT_NAMES = ("mat_id", "mat_n", "mat_s", "mat_w", "mat_e")
OFF_MATS = {nm: NFILT + i * P for i, nm in enumerate(MAT_NAMES)}
OFF_X = NFILT + 5 * P
NCBUF = OFF_X + NPLANE

# taps ordered center-(dy,dx) first so interior products can overlap halo fill
TAPS = sorted(
    ((dz, dy, dx) for dz in range(3) for dy in range(3) for dx in range(3)),
    key=lambda t: (abs(t[1] - 1) + abs(t[2] - 1), t[0]),
)


def _window(t, T):
    """Planes (0-indexed) written by step t that can still reach the readout."""
    lo = max(0, (D - 1) - (T - 1 - t))
    hi = min(D - 1, t + 1)
    return lo, hi


def _bcast_b(ap, n=BL):
    """Append a 0-stride broadcast dim of size n to an AP whose last dim is 1."""
    assert ap.ap[-1][1] == 1
    return AP(ap.tensor, ap.offset, ap.ap[:-1] + [[0, n]])


def build_program(T, legalize=True):
    nc = bass.Bass("TRN2", target_bir_lowering=False, debug=False)
    ind = nc.dram_tensor("inp", [P, NCBUF], F32, kind="ExternalInput").ap()
    outd = nc.dram_tensor("out", [P, NPLANE], F32, kind="ExternalOutput").ap()

    with tile.TileContext(nc) as tc:
        with ExitStack() as ctx:
            # Every tile is allocated exactly once (no pool rotation): the
            # TileScheduler's slot-reuse wait conditions bypass the vector
            # clock and can emit extra semaphore waits; plain RAW/WAR edges
            # on persistent tiles are clock-minimized instead.
            const = ctx.enter_context(tc.tile_pool(name="const", bufs=1))
            statep = ctx.enter_context(tc.tile_pool(name="state", bufs=1))
            psum = ctx.enter_context(tc.tile_pool(name="psum", bufs=1, space="PSUM"))

            cbuf = const.tile([P, NCBUF], F32)
            dma_in = nc.sync.dma_start(cbuf[:], ind)
            finals = [dma_in]  # per-proc final instructions, see tail NOPs
            mats = {nm: cbuf[:, off : off + P] for nm, off in OFF_MATS.items()}

            cps = psum.tile([P, 3 * 512], F32, tag="cps")
            mkr = psum.tile([P, 8], F32, tag="mkr")
            hps = {}
            for nm in ("mat_n", "mat_s", "mat_w", "mat_e"):
                htile = psum.tile([P, WMAX * WI * BL], F32, tag=f"hps_{nm}")
                hps[nm] = htile
            pgs = []
            for i in range(2):
                pgtile = statep.tile([P, GRP * WMAX * NPLANE + 8], F32, tag=f"pg{i}")
                pgs.append(pgtile)

            stA = statep.tile([P, NSTATE], F32, tag="stA")
            stB = statep.tile([P, NSTATE], F32, tag="stB")
            scr = statep.tile([P, 4], F32, tag="scr")  # DVE absorber dump
            scr2 = statep.tile([P, 4], F32, tag="scr2")  # ACT cover dump
            # Zero on DVE, then re-write in place on ACT: every state cell's
            # last writer is then ScalarE, so the loop's first-touch WAW deps
            # (halo copies, tanh, readout) stay single-semaphore.
            nc.vector.memset(stA[:], 0.0)
            nc.vector.memset(stB[:], 0.0)
            nc.scalar.copy(out=stA[:], in_=stA[:])
            nc.scalar.copy(out=stB[:], in_=stB[:])

            def st5(tile_):
                return tile_[:].rearrange(
                    "p (d h w b) -> p d h w b", d=DP, h=HPAD, w=WPAD, b=BL
                )

            ft5 = cbuf[:, :NFILT].rearrange(
                "p (d h w t) -> p d h w t", d=D, h=HI, w=WI, t=27
            )

            # x -> staging on DVE (waits the DMA) -> plane 0 interior on ACT
            xtmp = const.tile([P, NPLANE], F32, tag="xtmp")
            nc.vector.tensor_copy(out=xtmp[:], in_=cbuf[:, OFF_X:])
            nc.scalar.copy(
                out=st5(stA)[:, 1:2, 1 : 1 + HI, 1 : 1 + WI, :],
                in_=xtmp[:].rearrange("p (d h w b) -> p d h w b", d=1, h=HI, w=WI, b=BL),
            )

            # PE warmup: observe the input DMA once so later matmuls reading
            # mats/cbuf need no DMA wait; also initializes both marker cells
            # so the first absorbers read defined values
            nc.tensor.matmul(
                mkr[:, 0:1], mats["mat_id"], cbuf[:, 0:1], start=True, stop=True
            )
            nc.tensor.matmul(
                mkr[:, 1:2], mats["mat_id"], cbuf[:, 0:1], start=True, stop=True
            )

            cur, nxt = stA, stB
            gctr = 0  # global product-slot group counter
            for t in range(T):
                lo, hi = _window(t, T)
                assert lo <= hi
                wd = hi - lo + 1
                c5 = st5(cur)

                # ---- halo fill on `cur` for the planes written last step ----
                plo, phi = _window(t - 1, T) if t > 0 else (0, 0)
                np_ = phi - plo + 1
                dpi = plo + 1  # padded index of first halo plane
                # north: row hp=0 <- (hc-1)'s hp=3 ; south: hp=4 <- (hc+1)'s hp=1
                for nm, src_h, dst_h in (("mat_n", HI, 0), ("mat_s", 1, HPAD - 1)):
                    hp = hps[nm]
                    rhs = c5[:, dpi : dpi + np_, src_h : src_h + 1, 1 : 1 + WI, :]
                    nc.tensor.matmul(
                        hp[:, : np_ * WI * BL], mats[nm], rhs, start=True, stop=True
                    )
                    nc.scalar.copy(
                        out=c5[:, dpi : dpi + np_, dst_h : dst_h + 1, 1 : 1 + WI, :],
                        in_=hp[:, : np_ * WI * BL].rearrange(
                            "p (d o w b) -> p d o w b", d=np_, o=1, w=WI, b=BL
                        ),
                    )
                # west: col wp=0 <- (wc-1)'s wp=6 ; east: wp=7 <- (wc+1)'s wp=1
                # (after N/S so corner cells pick up diagonal neighbors)
                for nm, src_w, dst_w in (("mat_w", WI, 0), ("mat_e", 1, WPAD - 1)):
                    hp = hps[nm]
                    rhs = c5[:, dpi : dpi + np_, :, src_w : src_w + 1, :]
                    nc.tensor.matmul(
                        hp[:, : np_ * HPAD * BL], mats[nm], rhs, start=True, stop=True
                    )
                    nc.scalar.copy(
                        out=c5[:, dpi : dpi + np_, :, dst_w : dst_w + 1, :],
                        in_=hp[:, : np_ * HPAD * BL].rearrange(
                            "p (d h o b) -> p d h o b", d=np_, h=HPAD, o=1, b=BL
                        ),
                    )

                # ---- 27 products (DVE) + identity-matmul accumulate (PE) ----
                nchunks = (wd + 6) // 7  # <=7 planes (504 elems) per PSUM bank
                groups = [TAPS[i : i + GRP] for i in range(0, 27, GRP)]
                pg = None
                for grp in groups:
                    pg = pgs[gctr % 2]
                    # Absorb the PE tick of this buffer's previous readers
                    # (the matmuls of group gctr-2, via the marker stamped
                    # after them) so the products below need no PE wait. The
                    # copy lands on the first cell of each tap's range, which
                    # also gives the scheduler a WAW edge ordering it before
                    # the products.
                    mk_in = AP(
                        mkr[:, gctr % 2 : gctr % 2 + 1].tensor,
                        mkr[:, gctr % 2 : gctr % 2 + 1].offset,
                        mkr[:, gctr % 2 : gctr % 2 + 1].ap[:-1] + [[0, GRP]],
                    )
                    pg_dst = AP(
                        pg[:, 0:1].tensor,
                        pg[:, 0:1].offset,
                        pg[:, 0:1].ap[:-1] + [[WMAX * NPLANE, GRP]],
                    )
                    nc.vector.tensor_copy(out=pg_dst, in_=mk_in)
                    for gi, (dz, dy, dx) in enumerate(grp):
                        tap = (dz * 3 + dy) * 3 + dx
                        in0 = c5[
                            :, lo + dz : lo + dz + wd, dy : dy + HI, dx : dx + WI, :
                        ]
                        in1 = _bcast_b(ft5[:, lo : lo + wd, :, :, tap : tap + 1])
                        base = gi * WMAX * NPLANE
                        outp = pg[:, base : base + wd * NPLANE].rearrange(
                            "p (d h w b) -> p d h w b", d=wd, h=HI, w=WI, b=BL
                        )
                        last_prod = nc.vector.tensor_mul(out=outp, in0=in0, in1=in1)
                    first = grp is groups[0]
                    last = grp is groups[-1]
                    mms = []
                    for gi in range(len(grp)):
                        base = gi * WMAX * NPLANE
                        for c in range(nchunks):
                            pl = c * 7
                            pn = min(7, wd - pl)
                            mms.append(
                                nc.tensor.matmul(
                                    cps[:, c * 512 : c * 512 + pn * NPLANE],
                                    mats["mat_id"],
                                    pg[
                                        :,
                                        base + pl * NPLANE : base + (pl + pn) * NPLANE,
                                    ],
                                    start=(first and gi == 0),
                                    stop=(last and gi == len(grp) - 1),
                                )
                            )
                    # marker: stamps this group's last matmul tick into a PSUM
                    # cell the absorber two groups later will read; nosync
                    # edges pin it after the group's matmuls on the PE queue
                    marker = nc.tensor.matmul(
                        mkr[:, gctr % 2 : gctr % 2 + 1],
                        mats["mat_id"],
                        cbuf[:, 0:1],
                        start=True,
                        stop=True,
                    )
                    for mm in mms:
                        add_dep_helper(marker.ins, mm.ins, False, "marker order")
                    gctr += 1

                # hand the DVE clock to ScalarE: tanh's WAR on the previous
                # step's product reads then needs no extra wait (cell 1: the
                # absorber owns cell 0, the product of the last tap owns 1)
                last_cover = nc.scalar.copy(out=scr2[:, 0:1], in_=pg[:, 1:2])

                # ---- tanh (ACT) -> next state's interior ----
                n5 = st5(nxt)
                for c in range(nchunks):
                    pl = c * 7
                    pn = min(7, wd - pl)
                    last_tanh = nc.scalar.activation(
                        n5[:, lo + 1 + pl : lo + 1 + pl + pn, 1 : 1 + HI, 1 : 1 + WI, :],
                        cps[:, c * 512 : c * 512 + pn * NPLANE].rearrange(
                            "p (d h w b) -> p d h w b", d=pn, h=HI, w=WI, b=BL
                        ),
                        mybir.ActivationFunctionType.Tanh,
                    )
                cur, nxt = nxt, cur

            # ---- readout: volume plane D-1 (padded index D) interior ----
            dma_out = nc.sync.dma_start(
                outd.rearrange("p (d h w b) -> p d h w b", d=1, h=HI, w=WI, b=BL),
                st5(cur)[:, D : D + 1, 1 : 1 + HI, 1 : 1 + WI, :],
            )

            # Tail fan-in: one SP NOP per still-outstanding proc (each gets a
            # single wait). The Tile-generated tail drain on SP then re-waits
            # on the same final ticks; being later in the in-order SP stream
            # those waits are provably satisfied, so _legalize_waits drops
            # them (the drain would otherwise carry 5 waits; the limit is 1).
            finals += [last_tanh, last_cover, marker, last_prod, dma_out]
            for f in finals:
                nop = nc.sync.nop()
                add_dep_helper(nop.ins, f.ins, True, "tail fan-in")

    if legalize:
        # (CoreSim's race detector only credits semaphore sync, so validate
        # with legalize=False; the strip only removes same-engine waits that
        # in-order engine execution satisfies by construction.)
        _legalize_waits(nc)
    return nc


_ENGINE_SEM_PREFIX = {
    "EngineType.DVE": "DVE",
    "EngineType.Activation": "Activation",
    "EngineType.PE": "PE",
    "EngineType.Pool": "Pool",
    "EngineType.SP": "SP_sequencer",
}


def _legalize_waits(nc):
    """Enforce the 1-semaphore-wait-per-instruction limit of this walrus.

    1. Drop same-engine waits (engines execute their stream in order, and
       each engine's internal pipeline serializes an op's writes before the
       next op issues, so these are redundant).
    2. The kernel-tail drain aggregates one wait per outstanding proc; move
       the extras onto the other engines' drain instructions, whose own
       waits are trivial `>= 0` barrier placeholders. The tail barrier
       gathers all engines, so completion-before-end is preserved.
    """
    blocks = list(nc.m.functions[0].blocks)

    # (sem -> value) ticks already waited for by the SP stream (the body's
    # tail fan-in NOPs): the SP tail drain may drop waits at or below these.
    sp_covered = {}
    for b in blocks:
        for inst in b.instructions:
            si = inst.sync_info
            if si is None or not si.on_wait:
                continue
            if str(inst.engine) == "EngineType.SP":
                for w in si.on_wait:
                    if w.sync_type == "semaphore":
                        sp_covered[w.ant_name] = max(
                            sp_covered.get(w.ant_name, -1), w.wait_value
                        )

    for b in blocks:
        for inst in b.instructions:
            si = inst.sync_info
            if si is None or not si.on_wait:
                continue
            eng = _ENGINE_SEM_PREFIX.get(str(inst.engine))
            kept = [
                w
                for w in si.on_wait
                if not (
                    w.sync_type == "semaphore"
                    and eng is not None
                    and w.ant_name.rsplit("_", 1)[0] == eng
                )
            ]
            if len(kept) > 1 and type(inst).__name__ == "InstDrain":
                kept = [
                    w
                    for w in kept
                    if w.wait_value > sp_covered.get(w.ant_name, -1)
                ]
            si.on_wait = kept
            assert len(kept) <= 1, (
                f"{inst.name} {type(inst).__name__} on {inst.engine} still has "
                f"{[(w.ant_name, w.wait_value) for w in kept]}"
            )


def _host_inputs(x, filters):
    """Concatenated per-core device input [128, NCBUF] + the shared pieces."""
    f = np.ascontiguousarray(filters, dtype=np.float32)
    f = f.reshape(D, NHC, HI, NWC, WI, 27)
    f = f.transpose(1, 3, 0, 2, 4, 5).reshape(P, NFILT)

    mats = np.zeros((P, 5 * P), np.float32)
    eye = np.eye(P, dtype=np.float32)
    mats[:, 0:P] = eye
    mn = mats[:, P : 2 * P]
    ms = mats[:, 2 * P : 3 * P]
    mw = mats[:, 3 * P : 4 * P]
    me = mats[:, 4 * P : 5 * P]
    for m in range(P):
        hc, wc = divmod(m, NWC)
        if hc >= 1:
            mn[m - NWC, m] = 1.0  # psum[m] = rhs[m - NWC]
        if hc <= NHC - 2:
            ms[m + NWC, m] = 1.0
        if wc >= 1:
            mw[m - 1, m] = 1.0
        if wc <= NWC - 2:
            me[m + 1, m] = 1.0

    xf = np.ascontiguousarray(x, dtype=np.float32).reshape(NCORES, BL, H, W)
    ins = []
    for c in range(NCORES):
        xc = xf[c].reshape(BL, NHC, HI, NWC, WI)
        xc = xc.transpose(1, 3, 2, 4, 0).reshape(P, NPLANE)
        buf = np.concatenate([f, mats, xc], axis=1)
        assert buf.shape == (P, NCBUF)
        ins.append(np.ascontiguousarray(buf))
    return ins


def _pool_logits(plane):
    # plane: (B, H, W) fp32 -> (B, NUM_CLASSES) logits, matching reference
    row_mean = plane.mean(axis=2, dtype=np.float32)
    starts = [(i * H) // NUM_CLASSES for i in range(NUM_CLASSES)]
    ends = [-(-((i + 1) * H) // NUM_CLASSES) for i in range(NUM_CLASSES)]
    cols = [row_mean[:, s:e].mean(axis=1, dtype=np.float32) for s, e in zip(starts, ends)]
    return np.stack(cols, axis=1).astype(np.float32)


_PROG_CACHE = {}


def run_device(x, filters, T, trace=False):
    T = int(T)
    if T not in _PROG_CACHE:
        _PROG_CACHE[T] = build_program(T)
    nc = _PROG_CACHE[T]
    ins = _host_inputs(x, filters)
    in_maps = [{"inp": ins[c]} for c in range(NCORES)]
    res = run_bass_kernel_spmd(nc, in_maps, list(range(NCORES)), trace=trace)
    planes = []
    for c in range(NCORES):
        o = np.asarray(res.results[c]["out"]).reshape(NHC, NWC, HI, WI, BL)
        planes.append(o.transpose(4, 0, 2, 1, 3).reshape(BL, H, W))
    return np.concatenate(planes, axis=0), res


def kernel(x, filters, T):
    plane, _ = run_device(np.asarray(x), np.asarray(filters), T)
    return _pool_logits(plane)


# revision 39
# speedup vs baseline: 19.8623x; 19.8623x over previous
"""Trainium2 Bass kernel for DiffusionNet3D.

Math (see reference): state (B,48,48,48), T=64 steps of
  state = tanh(sum_{27 taps} shift(state) * per-voxel-filter),
then adaptive-pool the last depth plane into (B,10) logits.

Structure exploited:
  * state_0 is nonzero only in plane d=0 and the readout needs only plane
    d=47 at step T: at step t only planes [t-16, t+1] matter -> an <=18
    plane sliding window (3.6x compute cut vs the full 48-plane volume).
  * Spatially-varying weights make this elementwise work: DVE computes the
    27 shifted products; the TensorEngine accumulates them into PSUM with
    identity matmuls; ScalarE applies tanh.
  * Sharding: pure batch across the 8 cores (B=32 -> 4/core, no inter-core
    comm). On-chip: partitions = 128 (hc,wc) chunks of the 48x48 plane
    (3x6 each); free dim = (d-window, h, w, b) with d shifts as free-dim
    offsets. h/w halo cells are exchanged across partitions each step with
    shift-matrix matmuls on the otherwise idle PE (PSUM -> halo cells via
    ScalarE copies). Everything lives in SBUF; zero DMA inside the loop.

Toolchain constraint (axon/neuronxcc walrus): each instruction may carry at
most ONE semaphore wait. Tile emits one wait per dependency proc, including
redundant same-engine ones. The kernel is therefore structured so that every
instruction depends on exactly one foreign engine:
  * one concatenated input DMA (single DMA-queue semaphore),
  * "marker" matmuls + 1-elem DVE "absorber" reads convert the product-pool
    WAR (DVE write after PE read) into an already-observed PE tick,
  * a 1-elem ACT "cover" read of the last product each step hands the DVE
    clock to ScalarE so tanh's WAR on the previous step's products is free,
  * a post-pass strips the redundant same-engine waits (engines execute
    in order) and spreads the kernel-tail drain's waits across the other
    engines' drains (whose waits are trivial >=0 barrier placeholders).
"""

import numpy as np
from contextlib import ExitStack

import concourse.bass as bass
import concourse.mybir as mybir
import concourse.tile as tile
from concourse.bass import AP
from concourse.bass_utils import run_bass_kernel_spmd
from concourse.tile_rust import add_dep_helper

B, D, H, W = 32, 48, 48, 48
NUM_CLASSES = 10
NCORES = 8
BL = B // NCORES            # 4 batch per core
NHC, NWC = 16, 8            # h,w chunk grid -> 128 partitions
HI, WI = H // NHC, W // NWC  # 3, 6 chunk shape
P = NHC * NWC               # 128
DP, HPAD, WPAD = D + 2, HI + 2, WI + 2   # 50, 5, 8
NSTATE = DP * HPAD * WPAD * BL           # 8000 elems/partition
NPLANE = HI * WI * BL                    # 72 interior elems per plane
NFILT = D * HI * WI * 27                 # 23328 filter elems/partition
F32 = mybir.dt.float32

WMAX = 18                   # max active-window planes
GRP = 2                     # taps per product-pool slot
PGN = GRP * WMAX * NPLANE   # product slot payload elems (2592)

# cbuf layout: [filters | mat_id | mat_n | mat_s | mat_w | mat_e | x]
MAT_NAMES = ("mat_id", "mat_n", "mat_s", "mat_w", "mat_e")
OFF_MATS = {nm: NFILT + i * P for i, nm in enumerate(MAT_NAMES)}
OFF_X = NFILT + 5 * P
NCBUF = OFF_X + NPLANE

# taps ordered center-(dy,dx) first so interior products can overlap halo fill
TAPS = sorted(
    ((dz, dy, dx) for dz in range(3) for dy in range(3) for dx in range(3)),
    key=lambda t: (abs(t[1] - 1) + abs(t[2] - 1), t[0]),
)


def _window(t, T):
    """Planes (0-indexed) written by step t that can still reach the readout."""
    lo = max(0, (D - 1) - (T - 1 - t))
    hi = min(D - 1, t + 1)
    return lo, hi


def _bcast_b(ap, n=BL):
    """Append a 0-stride broadcast dim of size n to an AP whose last dim is 1."""
    assert ap.ap[-1][1] == 1
    return AP(ap.tensor, ap.offset, ap.ap[:-1] + [[0, n]])


def build_program(T, legalize=True, repeats=1):
    # repeats > 1 re-runs the whole T-step evolution on the evolving state
    # (output garbage) — used only to measure per-iteration device time
    # differentially, since the axon-tunnel call overhead is ~100 ms.
    nc = bass.Bass("TRN2", target_bir_lowering=False, debug=False)
    ind = nc.dram_tensor("inp", [P, NCBUF], F32, kind="ExternalInput").ap()
    outd = nc.dram_tensor("out", [P, NPLANE], F32, kind="ExternalOutput").ap()

    with tile.TileContext(nc) as tc:
        with ExitStack() as ctx:
            # Every tile is allocated exactly once (no pool rotation): the
            # TileScheduler's slot-reuse wait conditions bypass the vector
            # clock and can emit extra semaphore waits; plain RAW/WAR edges
            # on persistent tiles are clock-minimized instead.
            const = ctx.enter_context(tc.tile_pool(name="const", bufs=1))
            statep = ctx.enter_context(tc.tile_pool(name="state", bufs=1))
            psum = ctx.enter_context(tc.tile_pool(name="psum", bufs=1, space="PSUM"))

            cbuf = const.tile([P, NCBUF], F32)
            dma_in = nc.sync.dma_start(cbuf[:], ind)
            finals = [dma_in]  # per-proc final instructions, see tail NOPs
            mats = {nm: cbuf[:, off : off + P] for nm, off in OFF_MATS.items()}

            cps = psum.tile([P, 3 * 512], F32, tag="cps")
            mkr = psum.tile([P, 8], F32, tag="mkr")
            hps = {}
            for nm in ("mat_n", "mat_s", "mat_w", "mat_e"):
                htile = psum.tile([P, WMAX * WI * BL], F32, tag=f"hps_{nm}")
                hps[nm] = htile
            pgs = []
            for i in range(2):
                pgtile = statep.tile([P, GRP * WMAX * NPLANE + 8], F32, tag=f"pg{i}")
                pgs.append(pgtile)

            stA = statep.tile([P, NSTATE], F32, tag="stA")
            stB = statep.tile([P, NSTATE], F32, tag="stB")
            scr = statep.tile([P, 4], F32, tag="scr")  # DVE absorber dump
            scr2 = statep.tile([P, 4], F32, tag="scr2")  # ACT cover dump
            # Zero on DVE, then re-write in place on ACT: every state cell's
            # last writer is then ScalarE, so the loop's first-touch WAW deps
            # (halo copies, tanh, readout) stay single-semaphore.
            nc.vector.memset(stA[:], 0.0)
            nc.vector.memset(stB[:], 0.0)
            nc.scalar.copy(out=stA[:], in_=stA[:])
            nc.scalar.copy(out=stB[:], in_=stB[:])

            def st5(tile_):
                return tile_[:].rearrange(
                    "p (d h w b) -> p d h w b", d=DP, h=HPAD, w=WPAD, b=BL
                )

            ft5 = cbuf[:, :NFILT].rearrange(
                "p (d h w t) -> p d h w t", d=D, h=HI, w=WI, t=27
            )

            # x -> staging on DVE (waits the DMA) -> plane 0 interior on ACT
            xtmp = const.tile([P, NPLANE], F32, tag="xtmp")
            nc.vector.tensor_copy(out=xtmp[:], in_=cbuf[:, OFF_X:])
            nc.scalar.copy(
                out=st5(stA)[:, 1:2, 1 : 1 + HI, 1 : 1 + WI, :],
                in_=xtmp[:].rearrange("p (d h w b) -> p d h w b", d=1, h=HI, w=WI, b=BL),
            )

            # PE warmup: observe the input DMA once so later matmuls reading
            # mats/cbuf need no DMA wait; also initializes both marker cells
            # so the first absorbers read defined values
            nc.tensor.matmul(
                mkr[:, 0:1], mats["mat_id"], cbuf[:, 0:1], start=True, stop=True
            )
            nc.tensor.matmul(
                mkr[:, 1:2], mats["mat_id"], cbuf[:, 0:1], start=True, stop=True
            )

            cur, nxt = stA, stB
            gctr = 0  # global product-slot group counter
            for t in range(repeats * T):
                r, t = divmod(t, T)
                lo, hi = _window(t, T)
                assert lo <= hi
                wd = hi - lo + 1
                c5 = st5(cur)

                # ---- halo fill on `cur` for the planes written last step ----
                if t > 0:
                    plo, phi = _window(t - 1, T)
                elif r > 0:
                    plo, phi = _window(T - 1, T)
                else:
                    plo, phi = 0, 0
                np_ = phi - plo + 1
                dpi = plo + 1  # padded index of first halo plane
                # north: row hp=0 <- (hc-1)'s hp=3 ; south: hp=4 <- (hc+1)'s hp=1
                for nm, src_h, dst_h in (("mat_n", HI, 0), ("mat_s", 1, HPAD - 1)):
                    hp = hps[nm]
                    rhs = c5[:, dpi : dpi + np_, src_h : src_h + 1, 1 : 1 + WI, :]
                    nc.tensor.matmul(
                        hp[:, : np_ * WI * BL], mats[nm], rhs, start=True, stop=True
                    )
                    nc.scalar.copy(
                        out=c5[:, dpi : dpi + np_, dst_h : dst_h + 1, 1 : 1 + WI, :],
                        in_=hp[:, : np_ * WI * BL].rearrange(
                            "p (d o w b) -> p d o w b", d=np_, o=1, w=WI, b=BL
                        ),
                    )
                # west: col wp=0 <- (wc-1)'s wp=6 ; east: wp=7 <- (wc+1)'s wp=1
                # (after N/S so corner cells pick up diagonal neighbors)
                for nm, src_w, dst_w in (("mat_w", WI, 0), ("mat_e", 1, WPAD - 1)):
                    hp = hps[nm]
                    rhs = c5[:, dpi : dpi + np_, :, src_w : src_w + 1, :]
                    nc.tensor.matmul(
                        hp[:, : np_ * HPAD * BL], mats[nm], rhs, start=True, stop=True
                    )
                    nc.scalar.copy(
                        out=c5[:, dpi : dpi + np_, :, dst_w : dst_w + 1, :],
                        in_=hp[:, : np_ * HPAD * BL].rearrange(
                            "p (d h o b) -> p d h o b", d=np_, h=HPAD, o=1, b=BL
                        ),
                    )

                # ---- 27 products (DVE) + identity-matmul accumulate (PE) ----
                nchunks = (wd + 6) // 7  # <=7 planes (504 elems) per PSUM bank
                groups = [TAPS[i : i + GRP] for i in range(0, 27, GRP)]
                pg = None
                for grp in groups:
                    pg = pgs[gctr % 2]
                    # Absorb the PE tick of this buffer's previous readers
                    # (the matmuls of group gctr-2, via the marker stamped
                    # after them) so the products below need no PE wait. The
                    # copy lands on the first cell of each tap's range, which
                    # also gives the scheduler a WAW edge ordering it before
                    # the products.
                    mk_in = AP(
                        mkr[:, gctr % 2 : gctr % 2 + 1].tensor,
                        mkr[:, gctr % 2 : gctr % 2 + 1].offset,
                        mkr[:, gctr % 2 : gctr % 2 + 1].ap[:-1] + [[0, GRP]],
                    )
                    pg_dst = AP(
                        pg[:, 0:1].tensor,
                        pg[:, 0:1].offset,
                        pg[:, 0:1].ap[:-1] + [[WMAX * NPLANE, GRP]],
                    )
                    nc.vector.tensor_copy(out=pg_dst, in_=mk_in)
                    for gi, (dz, dy, dx) in enumerate(grp):
                        tap = (dz * 3 + dy) * 3 + dx
                        in0 = c5[
                            :, lo + dz : lo + dz + wd, dy : dy + HI, dx : dx + WI, :
                        ]
                        in1 = _bcast_b(ft5[:, lo : lo + wd, :, :, tap : tap + 1])
                        base = gi * WMAX * NPLANE
                        outp = pg[:, base : base + wd * NPLANE].rearrange(
                            "p (d h w b) -> p d h w b", d=wd, h=HI, w=WI, b=BL
                        )
                        last_prod = nc.vector.tensor_mul(out=outp, in0=in0, in1=in1)
                    first = grp is groups[0]
                    last = grp is groups[-1]
                    mms = []
                    for gi in range(len(grp)):
                        base = gi * WMAX * NPLANE
                        for c in range(nchunks):
                            pl = c * 7
                            pn = min(7, wd - pl)
                            mms.append(
                                nc.tensor.matmul(
                                    cps[:, c * 512 : c * 512 + pn * NPLANE],
                                    mats["mat_id"],
                                    pg[
                                        :,
                                        base + pl * NPLANE : base + (pl + pn) * NPLANE,
                                    ],
                                    start=(first and gi == 0),
                                    stop=(last and gi == len(grp) - 1),
                                )
                            )
                    # marker: stamps this group's last matmul tick into a PSUM
                    # cell the absorber two groups later will read; nosync
                    # edges pin it after the group's matmuls on the PE queue
                    marker = nc.tensor.matmul(
                        mkr[:, gctr % 2 : gctr % 2 + 1],
                        mats["mat_id"],
                        cbuf[:, 0:1],
                        start=True,
                        stop=True,
                    )
                    for mm in mms:
                        add_dep_helper(marker.ins, mm.ins, False, "marker order")
                    gctr += 1

                # hand the DVE clock to ScalarE: tanh's WAR on the previous
                # step's product reads then needs no extra wait (cell 1: the
                # absorber owns cell 0, the product of the last tap owns 1)
                last_cover = nc.scalar.copy(out=scr2[:, 0:1], in_=pg[:, 1:2])

                # ---- tanh (ACT) -> next state's interior ----
                n5 = st5(nxt)
                for c in range(nchunks):
                    pl = c * 7
                    pn = min(7, wd - pl)
                    last_tanh = nc.scalar.activation(
                        n5[:, lo + 1 + pl : lo + 1 + pl + pn, 1 : 1 + HI, 1 : 1 + WI, :],
                        cps[:, c * 512 : c * 512 + pn * NPLANE].rearrange(
                            "p (d h w b) -> p d h w b", d=pn, h=HI, w=WI, b=BL
                        ),
                        mybir.ActivationFunctionType.Tanh,
                    )
                cur, nxt = nxt, cur

            # ---- readout: volume plane D-1 (padded index D) interior ----
            dma_out = nc.sync.dma_start(
                outd.rearrange("p (d h w b) -> p d h w b", d=1, h=HI, w=WI, b=BL),
                st5(cur)[:, D : D + 1, 1 : 1 + HI, 1 : 1 + WI, :],
            )

            # Tail fan-in: one SP NOP per still-outstanding proc (each gets a
            # single wait). The Tile-generated tail drain on SP then re-waits
            # on the same final ticks; being later in the in-order SP stream
            # those waits are provably satisfied, so _legalize_waits drops
            # them (the drain would otherwise carry 5 waits; the limit is 1).
            finals += [last_tanh, last_cover, marker, last_prod, dma_out]
            for f in finals:
                nop = nc.sync.nop()
                add_dep_helper(nop.ins, f.ins, True, "tail fan-in")

    if legalize:
        # (CoreSim's race detector only credits semaphore sync, so validate
        # with legalize=False; the strip only removes same-engine waits that
        # in-order engine execution satisfies by construction.)
        _legalize_waits(nc)
    return nc


_ENGINE_SEM_PREFIX = {
    "EngineType.DVE": "DVE",
    "EngineType.Activation": "Activation",
    "EngineType.PE": "PE",
    "EngineType.Pool": "Pool",
    "EngineType.SP": "SP_sequencer",
}


def _legalize_waits(nc):
    """Enforce the 1-semaphore-wait-per-instruction limit of this walrus.

    1. Drop same-engine waits (engines execute their stream in order, and
       each engine's internal pipeline serializes an op's writes before the
       next op issues, so these are redundant).
    2. The kernel-tail drain aggregates one wait per outstanding proc; move
       the extras onto the other engines' drain instructions, whose own
       waits are trivial `>= 0` barrier placeholders. The tail barrier
       gathers all engines, so completion-before-end is preserved.
    """
    blocks = list(nc.m.functions[0].blocks)

    # (sem -> value) ticks already waited for by the SP stream (the body's
    # tail fan-in NOPs): the SP tail drain may drop waits at or below these.
    sp_covered = {}
    for b in blocks:
        for inst in b.instructions:
            si = inst.sync_info
            if si is None or not si.on_wait:
                continue
            if str(inst.engine) == "EngineType.SP":
                for w in si.on_wait:
                    if w.sync_type == "semaphore":
                        sp_covered[w.ant_name] = max(
                            sp_covered.get(w.ant_name, -1), w.wait_value
                        )

    for b in blocks:
        for inst in b.instructions:
            si = inst.sync_info
            if si is None or not si.on_wait:
                continue
            eng = _ENGINE_SEM_PREFIX.get(str(inst.engine))
            kept = [
                w
                for w in si.on_wait
                if not (
                    w.sync_type == "semaphore"
                    and eng is not None
                    and w.ant_name.rsplit("_", 1)[0] == eng
                )
            ]
            if len(kept) > 1 and type(inst).__name__ == "InstDrain":
                kept = [
                    w
                    for w in kept
                    if w.wait_value > sp_covered.get(w.ant_name, -1)
                ]
            si.on_wait = kept
            assert len(kept) <= 1, (
                f"{inst.name} {type(inst).__name__} on {inst.engine} still has "
                f"{[(w.ant_name, w.wait_value) for w in kept]}"
            )


def _host_inputs(x, filters):
    """Concatenated per-core device input [128, NCBUF] + the shared pieces."""
    f = np.ascontiguousarray(filters, dtype=np.float32)
    f = f.reshape(D, NHC, HI, NWC, WI, 27)
    f = f.transpose(1, 3, 0, 2, 4, 5).reshape(P, NFILT)

    mats = np.zeros((P, 5 * P), np.float32)
    eye = np.eye(P, dtype=np.float32)
    mats[:, 0:P] = eye
    mn = mats[:, P : 2 * P]
    ms = mats[:, 2 * P : 3 * P]
    mw = mats[:, 3 * P : 4 * P]
    me = mats[:, 4 * P : 5 * P]
    for m in range(P):
        hc, wc = divmod(m, NWC)
        if hc >= 1:
            mn[m - NWC, m] = 1.0  # psum[m] = rhs[m - NWC]
        if hc <= NHC - 2:
            ms[m + NWC, m] = 1.0
        if wc >= 1:
            mw[m - 1, m] = 1.0
        if wc <= NWC - 2:
            me[m + 1, m] = 1.0

    xf = np.ascontiguousarray(x, dtype=np.float32).reshape(NCORES, BL, H, W)
    ins = []
    for c in range(NCORES):
        xc = xf[c].reshape(BL, NHC, HI, NWC, WI)
        xc = xc.transpose(1, 3, 2, 4, 0).reshape(P, NPLANE)
        buf = np.concatenate([f, mats, xc], axis=1)
        assert buf.shape == (P, NCBUF)
        ins.append(np.ascontiguousarray(buf))
    return ins


def _pool_logits(plane):
    # plane: (B, H, W) fp32 -> (B, NUM_CLASSES) logits, matching reference
    row_mean = plane.mean(axis=2, dtype=np.float32)
    starts = [(i * H) // NUM_CLASSES for i in range(NUM_CLASSES)]
    ends = [-(-((i + 1) * H) // NUM_CLASSES) for i in range(NUM_CLASSES)]
    cols = [row_mean[:, s:e].mean(axis=1, dtype=np.float32) for s, e in zip(starts, ends)]
    return np.stack(cols, axis=1).astype(np.float32)


_PROG_CACHE = {}


def run_device(x, filters, T, trace=False):
    T = int(T)
    if T not in _PROG_CACHE:
        _PROG_CACHE[T] = build_program(T)
    nc = _PROG_CACHE[T]
    ins = _host_inputs(x, filters)
    in_maps = [{"inp": ins[c]} for c in range(NCORES)]
    res = run_bass_kernel_spmd(nc, in_maps, list(range(NCORES)), trace=trace)
    planes = []
    for c in range(NCORES):
        o = np.asarray(res.results[c]["out"]).reshape(NHC, NWC, HI, WI, BL)
        planes.append(o.transpose(4, 0, 2, 1, 3).reshape(BL, H, W))
    return np.concatenate(planes, axis=0), res


def kernel(x, filters, T):
    plane, _ = run_device(np.asarray(x), np.asarray(filters), T)
    return _pool_logits(plane)


# revision 45
# speedup vs baseline: 28.0635x; 1.4129x over previous
"""Trainium2 Bass kernel for DiffusionNet3D.

Math (see reference): state (B,48,48,48), T=64 steps of
  state = tanh(sum_{27 taps} shift(state) * per-voxel-filter),
then adaptive-pool the last depth plane into (B,10) logits.

Structure exploited:
  * state_0 is nonzero only in plane d=0 and the readout needs only plane
    d=47 at step T: at step t only planes [t-16, t+1] matter -> an <=18
    plane sliding window (3.6x compute cut vs the full 48-plane volume).
  * Spatially-varying weights make this elementwise work: DVE computes the
    27 shifted products; the TensorEngine accumulates them into PSUM with
    identity matmuls; ScalarE applies tanh.
  * Sharding: pure batch across the 8 cores (B=32 -> 4/core, no inter-core
    comm). On-chip: partitions = 128 (hc,wc) chunks of the 48x48 plane
    (3x6 each); free dim = (d-window, h, w, b) with d shifts as free-dim
    offsets. h/w halo cells are exchanged across partitions each step with
    shift-matrix matmuls on the otherwise idle PE (PSUM -> halo cells via
    ScalarE copies). Everything lives in SBUF; zero DMA inside the loop.

Toolchain constraint (axon/neuronxcc walrus): each instruction may carry at
most ONE semaphore wait. Tile emits one wait per dependency proc, including
redundant same-engine ones. The kernel is therefore structured so that every
instruction depends on exactly one foreign engine:
  * one concatenated input DMA (single DMA-queue semaphore),
  * "marker" matmuls + 1-elem DVE "absorber" reads convert the product-pool
    WAR (DVE write after PE read) into an already-observed PE tick,
  * a 1-elem ACT "cover" read of the last product each step hands the DVE
    clock to ScalarE so tanh's WAR on the previous step's products is free,
  * a post-pass strips the redundant same-engine waits (engines execute
    in order) and spreads the kernel-tail drain's waits across the other
    engines' drains (whose waits are trivial >=0 barrier placeholders).
"""

import numpy as np
from contextlib import ExitStack

import concourse.bass as bass
import concourse.mybir as mybir
import concourse.tile as tile
from concourse.bass import AP
from concourse.bass_utils import run_bass_kernel_spmd
from concourse.tile_rust import add_dep_helper

B, D, H, W = 32, 48, 48, 48
NUM_CLASSES = 10
NCORES = 8
BL = B // NCORES            # 4 batch per core
NHC, NWC = 16, 8            # h,w chunk grid -> 128 partitions
HI, WI = H // NHC, W // NWC  # 3, 6 chunk shape
P = NHC * NWC               # 128
DP, HPAD, WPAD = D + 2, HI + 2, WI + 2   # 50, 5, 8
NSTATE = DP * HPAD * WPAD * BL           # 8000 elems/partition
NPLANE = HI * WI * BL                    # 72 interior elems per plane
NFILT = D * HI * WI * 27                 # 23328 filter elems/partition
F32 = mybir.dt.float32
BF16 = mybir.dt.bfloat16
# Products are rounded to bf16 before the fp32 PSUM accumulation: the PE
# streams bf16 moving operands ~10x faster than fp32 (which runs multi-pass),
# and only the 27 summands lose precision — state/filters/halos stay fp32.
PROD_DT = mybir.dt.float16

WMAX = 18                   # max active-window planes
GRP = 2                     # taps per product-pool slot
PGN = GRP * WMAX * NPLANE   # product slot payload elems (2592)

# cbuf layout: [filters | mat_id | mat_n | mat_s | mat_w | mat_e | x]
MAT_NAMES = ("mat_id", "mat_n", "mat_s", "mat_w", "mat_e")
OFF_MATS = {nm: NFILT + i * P for i, nm in enumerate(MAT_NAMES)}
OFF_X = NFILT + 5 * P
NCBUF = OFF_X + NPLANE

# taps ordered center-(dy,dx) first so interior products can overlap halo fill
TAPS = sorted(
    ((dz, dy, dx) for dz in range(3) for dy in range(3) for dx in range(3)),
    key=lambda t: (abs(t[1] - 1) + abs(t[2] - 1), t[0]),
)


def _window(t, T):
    """Planes (0-indexed) written by step t that can still reach the readout."""
    lo = max(0, (D - 1) - (T - 1 - t))
    hi = min(D - 1, t + 1)
    return lo, hi


def _bcast_b(ap, n=BL):
    """Append a 0-stride broadcast dim of size n to an AP whose last dim is 1."""
    assert ap.ap[-1][1] == 1
    return AP(ap.tensor, ap.offset, ap.ap[:-1] + [[0, n]])


def build_program(T, legalize=True, repeats=1):
    # repeats > 1 re-runs the whole T-step evolution on the evolving state
    # (output garbage) — used only to measure per-iteration device time
    # differentially, since the axon-tunnel call overhead is ~100 ms.
    nc = bass.Bass("TRN2", target_bir_lowering=False, debug=False)
    ind = nc.dram_tensor("inp", [P, NCBUF], F32, kind="ExternalInput").ap()
    outd = nc.dram_tensor("out", [P, NPLANE], F32, kind="ExternalOutput").ap()

    with tile.TileContext(nc) as tc:
        with ExitStack() as ctx:
            # Every tile is allocated exactly once (no pool rotation): the
            # TileScheduler's slot-reuse wait conditions bypass the vector
            # clock and can emit extra semaphore waits; plain RAW/WAR edges
            # on persistent tiles are clock-minimized instead.
            const = ctx.enter_context(tc.tile_pool(name="const", bufs=1))
            statep = ctx.enter_context(tc.tile_pool(name="state", bufs=1))
            psum = ctx.enter_context(tc.tile_pool(name="psum", bufs=1, space="PSUM"))

            cbuf = const.tile([P, NCBUF], F32)
            dma_in = nc.sync.dma_start(cbuf[:], ind)
            finals = [dma_in]  # per-proc final instructions, see tail NOPs
            mats = {nm: cbuf[:, off : off + P] for nm, off in OFF_MATS.items()}

            cps = psum.tile([P, 3 * 512], F32, tag="cps")
            mkr = psum.tile([P, 8], F32, tag="mkr")
            hps = {}
            for nm in ("mat_n", "mat_s", "mat_w", "mat_e"):
                htile = psum.tile([P, WMAX * WI * BL], F32, tag=f"hps_{nm}")
                hps[nm] = htile
            pgs = []
            for i in range(2):
                pgtile = statep.tile(
                    [P, GRP * WMAX * NPLANE + 8], PROD_DT, tag=f"pg{i}"
                )
                pgs.append(pgtile)
            # identity in the product dtype for the accumulation matmuls
            idp = statep.tile([P, P], PROD_DT, tag="idp")

            stA = statep.tile([P, NSTATE], F32, tag="stA")
            stB = statep.tile([P, NSTATE], F32, tag="stB")
            scr = statep.tile([P, 4], F32, tag="scr")  # DVE absorber dump
            scr2 = statep.tile([P, 4], F32, tag="scr2")  # ACT cover dump
            # Zero on DVE, then re-write in place on ACT: every state cell's
            # last writer is then ScalarE, so the loop's first-touch WAW deps
            # (halo copies, tanh, readout) stay single-semaphore.
            nc.vector.memset(stA[:], 0.0)
            nc.vector.memset(stB[:], 0.0)
            nc.scalar.copy(out=stA[:], in_=stA[:])
            nc.scalar.copy(out=stB[:], in_=stB[:])

            def st5(tile_):
                return tile_[:].rearrange(
                    "p (d h w b) -> p d h w b", d=DP, h=HPAD, w=WPAD, b=BL
                )

            ft5 = cbuf[:, :NFILT].rearrange(
                "p (d h w t) -> p d h w t", d=D, h=HI, w=WI, t=27
            )

            # x -> staging on DVE (waits the DMA) -> plane 0 interior on ACT
            xtmp = const.tile([P, NPLANE], F32, tag="xtmp")
            nc.vector.tensor_copy(out=xtmp[:], in_=cbuf[:, OFF_X:])
            nc.vector.tensor_copy(out=idp[:], in_=mats["mat_id"])
            nc.scalar.copy(
                out=st5(stA)[:, 1:2, 1 : 1 + HI, 1 : 1 + WI, :],
                in_=xtmp[:].rearrange("p (d h w b) -> p d h w b", d=1, h=HI, w=WI, b=BL),
            )

            # PE warmup: observe the input DMA once so later matmuls reading
            # mats/cbuf need no DMA wait; also initializes both marker cells
            # so the first absorbers read defined values
            nc.tensor.matmul(
                mkr[:, 0:1], mats["mat_id"], cbuf[:, 0:1], start=True, stop=True
            )
            nc.tensor.matmul(
                mkr[:, 1:2], mats["mat_id"], cbuf[:, 0:1], start=True, stop=True
            )

            cur, nxt = stA, stB
            gctr = 0  # global product-slot group counter
            for t in range(repeats * T):
                r, t = divmod(t, T)
                lo, hi = _window(t, T)
                assert lo <= hi
                wd = hi - lo + 1
                c5 = st5(cur)

                # ---- halo fill on `cur` for the planes written last step ----
                if t > 0:
                    plo, phi = _window(t - 1, T)
                elif r > 0:
                    plo, phi = _window(T - 1, T)
                else:
                    plo, phi = 0, 0
                np_ = phi - plo + 1
                dpi = plo + 1  # padded index of first halo plane
                # north: row hp=0 <- (hc-1)'s hp=3 ; south: hp=4 <- (hc+1)'s hp=1
                for nm, src_h, dst_h in (("mat_n", HI, 0), ("mat_s", 1, HPAD - 1)):
                    hp = hps[nm]
                    rhs = c5[:, dpi : dpi + np_, src_h : src_h + 1, 1 : 1 + WI, :]
                    nc.tensor.matmul(
                        hp[:, : np_ * WI * BL], mats[nm], rhs, start=True, stop=True
                    )
                    nc.scalar.copy(
                        out=c5[:, dpi : dpi + np_, dst_h : dst_h + 1, 1 : 1 + WI, :],
                        in_=hp[:, : np_ * WI * BL].rearrange(
                            "p (d o w b) -> p d o w b", d=np_, o=1, w=WI, b=BL
                        ),
                    )
                # west: col wp=0 <- (wc-1)'s wp=6 ; east: wp=7 <- (wc+1)'s wp=1
                # (after N/S so corner cells pick up diagonal neighbors)
                for nm, src_w, dst_w in (("mat_w", WI, 0), ("mat_e", 1, WPAD - 1)):
                    hp = hps[nm]
                    rhs = c5[:, dpi : dpi + np_, :, src_w : src_w + 1, :]
                    nc.tensor.matmul(
                        hp[:, : np_ * HPAD * BL], mats[nm], rhs, start=True, stop=True
                    )
                    nc.scalar.copy(
                        out=c5[:, dpi : dpi + np_, :, dst_w : dst_w + 1, :],
                        in_=hp[:, : np_ * HPAD * BL].rearrange(
                            "p (d h o b) -> p d h o b", d=np_, h=HPAD, o=1, b=BL
                        ),
                    )

                # ---- 27 products (DVE) + identity-matmul accumulate (PE) ----
                nchunks = (wd + 6) // 7  # <=7 planes (504 elems) per PSUM bank
                groups = [TAPS[i : i + GRP] for i in range(0, 27, GRP)]
                pg = None
                for grp in groups:
                    pg = pgs[gctr % 2]
                    # Absorb the PE tick of this buffer's previous readers
                    # (the matmuls of group gctr-2, via the marker stamped
                    # after them) so the products below need no PE wait. The
                    # copy lands on the first cell of each tap's range, which
                    # also gives the scheduler a WAW edge ordering it before
                    # the products.
                    mk_in = AP(
                        mkr[:, gctr % 2 : gctr % 2 + 1].tensor,
                        mkr[:, gctr % 2 : gctr % 2 + 1].offset,
                        mkr[:, gctr % 2 : gctr % 2 + 1].ap[:-1] + [[0, GRP]],
                    )
                    pg_dst = AP(
                        pg[:, 0:1].tensor,
                        pg[:, 0:1].offset,
                        pg[:, 0:1].ap[:-1] + [[WMAX * NPLANE, GRP]],
                    )
                    nc.vector.tensor_copy(out=pg_dst, in_=mk_in)
                    for gi, (dz, dy, dx) in enumerate(grp):
                        tap = (dz * 3 + dy) * 3 + dx
                        in0 = c5[
                            :, lo + dz : lo + dz + wd, dy : dy + HI, dx : dx + WI, :
                        ]
                        in1 = _bcast_b(ft5[:, lo : lo + wd, :, :, tap : tap + 1])
                        base = gi * WMAX * NPLANE
                        outp = pg[:, base : base + wd * NPLANE].rearrange(
                            "p (d h w b) -> p d h w b", d=wd, h=HI, w=WI, b=BL
                        )
                        last_prod = nc.vector.tensor_mul(out=outp, in0=in0, in1=in1)
                    first = grp is groups[0]
                    last = grp is groups[-1]
                    mms = []
                    for gi in range(len(grp)):
                        base = gi * WMAX * NPLANE
                        for c in range(nchunks):
                            pl = c * 7
                            pn = min(7, wd - pl)
                            mms.append(
                                nc.tensor.matmul(
                                    cps[:, c * 512 : c * 512 + pn * NPLANE],
                                    idp[:],
                                    pg[
                                        :,
                                        base + pl * NPLANE : base + (pl + pn) * NPLANE,
                                    ],
                                    start=(first and gi == 0),
                                    stop=(last and gi == len(grp) - 1),
                                )
                            )
                    # marker: stamps this group's last matmul tick into a PSUM
                    # cell the absorber two groups later will read; nosync
                    # edges pin it after the group's matmuls on the PE queue
                    marker = nc.tensor.matmul(
                        mkr[:, gctr % 2 : gctr % 2 + 1],
                        idp[:],
                        idp[:, 0:1],
                        start=True,
                        stop=True,
                    )
                    for mm in mms:
                        add_dep_helper(marker.ins, mm.ins, False, "marker order")
                    gctr += 1

                # hand the DVE clock to ScalarE: tanh's WAR on the previous
                # step's product reads then needs no extra wait (cell 1: the
                # absorber owns cell 0, the product of the last tap owns 1)
                last_cover = nc.scalar.copy(out=scr2[:, 0:1], in_=pg[:, 1:2])

                # ---- tanh (ACT) -> next state's interior ----
                n5 = st5(nxt)
                for c in range(nchunks):
                    pl = c * 7
                    pn = min(7, wd - pl)
                    last_tanh = nc.scalar.activation(
                        n5[:, lo + 1 + pl : lo + 1 + pl + pn, 1 : 1 + HI, 1 : 1 + WI, :],
                        cps[:, c * 512 : c * 512 + pn * NPLANE].rearrange(
                            "p (d h w b) -> p d h w b", d=pn, h=HI, w=WI, b=BL
                        ),
                        mybir.ActivationFunctionType.Tanh,
                    )
                cur, nxt = nxt, cur

            # ---- readout: volume plane D-1 (padded index D) interior ----
            dma_out = nc.sync.dma_start(
                outd.rearrange("p (d h w b) -> p d h w b", d=1, h=HI, w=WI, b=BL),
                st5(cur)[:, D : D + 1, 1 : 1 + HI, 1 : 1 + WI, :],
            )

            # Tail fan-in: one SP NOP per still-outstanding proc (each gets a
            # single wait). The Tile-generated tail drain on SP then re-waits
            # on the same final ticks; being later in the in-order SP stream
            # those waits are provably satisfied, so _legalize_waits drops
            # them (the drain would otherwise carry 5 waits; the limit is 1).
            finals += [last_tanh, last_cover, marker, last_prod, dma_out]
            for f in finals:
                nop = nc.sync.nop()
                add_dep_helper(nop.ins, f.ins, True, "tail fan-in")

    if legalize:
        # (CoreSim's race detector only credits semaphore sync, so validate
        # with legalize=False; the strip only removes same-engine waits that
        # in-order engine execution satisfies by construction.)
        _legalize_waits(nc)
    return nc


_ENGINE_SEM_PREFIX = {
    "EngineType.DVE": "DVE",
    "EngineType.Activation": "Activation",
    "EngineType.PE": "PE",
    "EngineType.Pool": "Pool",
    "EngineType.SP": "SP_sequencer",
}


def _legalize_waits(nc):
    """Enforce the 1-semaphore-wait-per-instruction limit of this walrus.

    1. Drop same-engine waits (engines execute their stream in order, and
       each engine's internal pipeline serializes an op's writes before the
       next op issues, so these are redundant).
    2. The kernel-tail drain aggregates one wait per outstanding proc; move
       the extras onto the other engines' drain instructions, whose own
       waits are trivial `>= 0` barrier placeholders. The tail barrier
       gathers all engines, so completion-before-end is preserved.
    """
    blocks = list(nc.m.functions[0].blocks)

    # (sem -> value) ticks already waited for by the SP stream (the body's
    # tail fan-in NOPs): the SP tail drain may drop waits at or below these.
    sp_covered = {}
    for b in blocks:
        for inst in b.instructions:
            si = inst.sync_info
            if si is None or not si.on_wait:
                continue
            if str(inst.engine) == "EngineType.SP":
                for w in si.on_wait:
                    if w.sync_type == "semaphore":
                        sp_covered[w.ant_name] = max(
                            sp_covered.get(w.ant_name, -1), w.wait_value
                        )

    for b in blocks:
        for inst in b.instructions:
            si = inst.sync_info
            if si is None or not si.on_wait:
                continue
            eng = _ENGINE_SEM_PREFIX.get(str(inst.engine))
            kept = [
                w
                for w in si.on_wait
                if not (
                    w.sync_type == "semaphore"
                    and eng is not None
                    and w.ant_name.rsplit("_", 1)[0] == eng
                )
            ]
            if len(kept) > 1 and type(inst).__name__ == "InstDrain":
                kept = [
                    w
                    for w in kept
                    if w.wait_value > sp_covered.get(w.ant_name, -1)
                ]
            si.on_wait = kept
            assert len(kept) <= 1, (
                f"{inst.name} {type(inst).__name__} on {inst.engine} still has "
                f"{[(w.ant_name, w.wait_value) for w in kept]}"
            )


def _host_inputs(x, filters):
    """Concatenated per-core device input [128, NCBUF] + the shared pieces."""
    f = np.ascontiguousarray(filters, dtype=np.float32)
    f = f.reshape(D, NHC, HI, NWC, WI, 27)
    f = f.transpose(1, 3, 0, 2, 4, 5).reshape(P, NFILT)

    mats = np.zeros((P, 5 * P), np.float32)
    eye = np.eye(P, dtype=np.float32)
    mats[:, 0:P] = eye
    mn = mats[:, P : 2 * P]
    ms = mats[:, 2 * P : 3 * P]
    mw = mats[:, 3 * P : 4 * P]
    me = mats[:, 4 * P : 5 * P]
    for m in range(P):
        hc, wc = divmod(m, NWC)
        if hc >= 1:
            mn[m - NWC, m] = 1.0  # psum[m] = rhs[m - NWC]
        if hc <= NHC - 2:
            ms[m + NWC, m] = 1.0
        if wc >= 1:
            mw[m - 1, m] = 1.0
        if wc <= NWC - 2:
            me[m + 1, m] = 1.0

    xf = np.ascontiguousarray(x, dtype=np.float32).reshape(NCORES, BL, H, W)
    ins = []
    for c in range(NCORES):
        xc = xf[c].reshape(BL, NHC, HI, NWC, WI)
        xc = xc.transpose(1, 3, 2, 4, 0).reshape(P, NPLANE)
        buf = np.concatenate([f, mats, xc], axis=1)
        assert buf.shape == (P, NCBUF)
        ins.append(np.ascontiguousarray(buf))
    return ins


def _pool_logits(plane):
    # plane: (B, H, W) fp32 -> (B, NUM_CLASSES) logits, matching reference
    row_mean = plane.mean(axis=2, dtype=np.float32)
    starts = [(i * H) // NUM_CLASSES for i in range(NUM_CLASSES)]
    ends = [-(-((i + 1) * H) // NUM_CLASSES) for i in range(NUM_CLASSES)]
    cols = [row_mean[:, s:e].mean(axis=1, dtype=np.float32) for s, e in zip(starts, ends)]
    return np.stack(cols, axis=1).astype(np.float32)


_PROG_CACHE = {}


def run_device(x, filters, T, trace=False):
    T = int(T)
    if T not in _PROG_CACHE:
        _PROG_CACHE[T] = build_program(T)
    nc = _PROG_CACHE[T]
    ins = _host_inputs(x, filters)
    in_maps = [{"inp": ins[c]} for c in range(NCORES)]
    res = run_bass_kernel_spmd(nc, in_maps, list(range(NCORES)), trace=trace)
    planes = []
    for c in range(NCORES):
        o = np.asarray(res.results[c]["out"]).reshape(NHC, NWC, HI, WI, BL)
        planes.append(o.transpose(4, 0, 2, 1, 3).reshape(BL, H, W))
    return np.concatenate(planes, axis=0), res


def kernel(x, filters, T):
    plane, _ = run_device(np.asarray(x), np.asarray(filters), T)
    return _pool_logits(plane)
